# revision 21
# baseline (speedup 1.0000x reference)
"""CondConv2d (MoE routed conv) Trainium2 Bass kernel.

Strategy
--------
Data-parallel over batch B=32 across 8 NeuronCores (4 samples/core); the
expert bank + routing params are replicated.  Per core and sample:

  1. x[b] is DMA'd into SBUF as two zero-padded [128, 58*58] tiles
     (C=256 split across 2 partition chunks, H/W padded by 1).
  2. pooled = sum_hw(x)  (DVE free-dim reduce; pad zeros don't matter).
     Routing MLP (relu + softmax over E=4) runs on PE/ACT/DVE; the 4
     routing scalars are partition-broadcast via GPSIMD.
  3. Expert mixing: combined[c,(ij,o)] = sum_e r_e * experts[c,e,(ij,o)]
     as a fused scalar_tensor_tensor chain on DVE.  experts are
     host-relayout'ed to [C, E, 3*3, O] so the SBUF slabs DMA contiguously
     and mixed tiles are directly in matmul-lhsT orientation.
  4. Conv = 9 shifted 1x1 convs accumulated in PSUM: for each output
     chunk [128 o, 448] (8 rows x 56 cols), accumulate 2 (c-chunk) x 9
     (shift) matmuls, N=448, operands bitcast to float32r (full PE rate
     at fp32 storage).
  5. PSUM -> SBUF copies on ACT, then HWDGE DMA to HBM.
"""

import numpy as np
from contextlib import ExitStack

import concourse.bass as bass
import concourse.bacc as bacc
import concourse.mybir as mybir
import concourse.tile as tile
from concourse.bass_utils import run_bass_kernel_spmd

F32 = mybir.dt.float32
F32R = mybir.dt.float32r
AF = mybir.ActivationFunctionType
ALU = mybir.AluOpType
AX = mybir.AxisListType

# Problem shapes (hardcoded per contract).
B, C, H, W = 32, 256, 56, 56
E, O, K = 4, 256, 3
HID = 64
NCORES = 8
BL = B // NCORES          # samples per core
CCH = C // 128            # c partition chunks
OCH = O // 128            # o partition chunks
HP, WP = H + 2, W + 2     # padded
RB_ROWS = 8               # output rows per matmul
NRB = H // RB_ROWS        # 7 row blocks
NBLK = RB_ROWS * W        # 448 = matmul free size
KK = K * K

_CACHE = {}


def _build_program(use_f32r=True, reps=1, loop_n=None):
    nc = bacc.Bacc("TRN2", target_bir_lowering=False, debug=False)

    x_d = nc.dram_tensor("x", [BL, C, HP, WP], F32, kind="ExternalInput").ap()
    ex_d = nc.dram_tensor("experts_t", [C, E, KK, O], F32, kind="ExternalInput").ap()
    rw1t_d = nc.dram_tensor("rw1t", [C, HID], F32, kind="ExternalInput").ap()
    rb1_d = nc.dram_tensor("rb1", [HID, 1], F32, kind="ExternalInput").ap()
    rw2t_d = nc.dram_tensor("rw2t", [HID, E], F32, kind="ExternalInput").ap()
    rb2_d = nc.dram_tensor("rb2", [1, E], F32, kind="ExternalInput").ap()
    out_d = nc.dram_tensor("out", [BL, O, H, W], F32, kind="ExternalOutput").ap()

    mmdt = F32R if use_f32r else F32

    with tile.TileContext(nc) as tc, ExitStack() as ctx:
        const_pool = ctx.enter_context(tc.tile_pool(name="const", bufs=1))
        xpad_pool = ctx.enter_context(tc.tile_pool(name="xpad", bufs=2 * CCH))
        comb_pool = ctx.enter_context(tc.tile_pool(name="comb", bufs=2 * CCH))
        scr_pool = ctx.enter_context(tc.tile_pool(name="scr", bufs=2))
        ostg_pool = ctx.enter_context(tc.tile_pool(name="ostg", bufs=4))
        small_pool = ctx.enter_context(tc.tile_pool(name="small", bufs=2))
        cpsum_pool = ctx.enter_context(tc.tile_pool(name="cpsum", bufs=NRB, space="PSUM"))
        mpsum_pool = ctx.enter_context(tc.tile_pool(name="mpsum", bufs=1, space="PSUM"))

        # ---- constants / parameters (preload once) ----
        rw1t_t = []
        for cc in range(CCH):
            t = const_pool.tile([128, HID], F32, name=f"rw1t{cc}")
            nc.sync.dma_start(t[:], rw1t_d[cc * 128:(cc + 1) * 128, :])
            rw1t_t.append(t)
        rb1_t = const_pool.tile([HID, 1], F32, name="rb1")
        nc.sync.dma_start(rb1_t[:], rb1_d[:])
        rw2t_t = const_pool.tile([HID, E], F32, name="rw2t")
        nc.sync.dma_start(rw2t_t[:], rw2t_d[:])
        rb2_t = const_pool.tile([1, E], F32, name="rb2")
        nc.sync.dma_start(rb2_t[:], rb2_d[:])
        ones_t = const_pool.tile([1, 128], F32, name="ones")
        nc.vector.memset(ones_t[:], 1.0)

        slabs = []
        for cc in range(CCH):
            t = const_pool.tile([128, E * KK * O], F32, name=f"slab{cc}")
            nc.sync.dma_start(t[:], ex_d[cc * 128:(cc + 1) * 128])
            slabs.append(t)

        # per-sample state
        xv = {}       # (b, cc) -> padded x tile viewed [128, HP, WP]
        comb = {}     # (b, cc) -> combined weights [128, KK*O]

        def emit_loads(b):
            for cc in range(CCH):
                t = xpad_pool.tile([128, HP * WP], mmdt, tag="xpad",
                                   name=f"xp{b}_{cc}")
                # x is host-pre-padded; SWDGE path casts f32 -> f32r
                # (rounding) during the DMA.
                dma_eng = nc.gpsimd if use_f32r else nc.sync
                dma_eng.dma_start(t[:], x_d[b, cc * 128:(cc + 1) * 128])
                xv[(b, cc)] = t.rearrange("p (h w) -> p h w", w=WP)

        def emit_routing(b):
            pooled = []
            for cc in range(CCH):
                p = small_pool.tile([128, 1], F32, tag="pooled", bufs=4,
                                    name=f"pool{b}_{cc}")
                nc.vector.reduce_sum(out=p[:], in_=xv[(b, cc)][:], axis=AX.XY)
                pooled.append(p)
            mps = mpsum_pool.tile([128, 512], F32, tag="mps", name=f"mps{b}")
            for cc in range(CCH):
                nc.tensor.matmul(mps[0:HID, 0:1], rw1t_t[cc][:], pooled[cc][:],
                                 start=(cc == 0), stop=(cc == CCH - 1))
            h_sb = small_pool.tile([HID, 1], F32, tag="h", name=f"h{b}")
            nc.scalar.activation(h_sb[:], mps[0:HID, 0:1], AF.Relu, bias=rb1_t[:])
            nc.tensor.matmul(mps[0:1, 4:4 + E], h_sb[:], rw2t_t[:],
                             start=True, stop=True)
            ze = small_pool.tile([1, E], F32, tag="ze", name=f"ze{b}")
            nc.vector.tensor_add(ze[:], mps[0:1, 4:4 + E], rb2_t[:])
            es = small_pool.tile([1, E], F32, tag="es", name=f"es{b}")
            nc.scalar.activation(es[:], ze[:], AF.Exp)
            ssum = small_pool.tile([1, 1], F32, tag="ssum", name=f"ss{b}")
            nc.vector.reduce_sum(out=ssum[:], in_=es[:], axis=AX.X)
            rec = small_pool.tile([1, 1], F32, tag="rec", name=f"rec{b}")
            nc.vector.reciprocal(rec[:], ssum[:])
            r_t = small_pool.tile([1, E], F32, tag="r", name=f"r{b}")
            nc.vector.tensor_scalar_mul(r_t[:], es[:], rec[:])
            # broadcast r across partitions: ones[1,128].T @ r[1,E] on PE
            nc.tensor.matmul(mps[0:128, 8:8 + E], ones_t[:], r_t[:],
                             start=True, stop=True)
            rbc = small_pool.tile([128, E], F32, tag="rbc", name=f"rbc{b}")
            nc.scalar.copy(rbc[:], mps[0:128, 8:8 + E])
            return rbc

        def emit_mixing(b, rbc):
            S = E * KK * O // CCH * CCH  # noqa (slab free size is E*KK*O)
            seg = KK * O
            for cc in range(CCH):
                slab = slabs[cc]
                a = scr_pool.tile([128, seg], F32, tag="scr", name=f"scr{b}_{cc}")
                nc.vector.tensor_scalar_mul(a[:], slab[:, 0:seg], rbc[:, 0:1])
                for e in range(1, E - 1):
                    nc.vector.scalar_tensor_tensor(
                        a[:], slab[:, e * seg:(e + 1) * seg], rbc[:, e:e + 1],
                        a[:], op0=ALU.mult, op1=ALU.add)
                cmb = comb_pool.tile([128, seg], mmdt, tag="comb",
                                     name=f"cmb{b}_{cc}")
                nc.vector.scalar_tensor_tensor(
                    cmb[:], slab[:, (E - 1) * seg:E * seg], rbc[:, E - 1:E],
                    a[:], op0=ALU.mult, op1=ALU.add)
                comb[(b, cc)] = cmb

        def emit_conv_ochunk(b, oc):
            ptiles = [cpsum_pool.tile([128, NBLK], F32, tag="cps",
                                      name=f"cp{b}_{oc}_{rb}")
                      for rb in range(NRB)]
            for cc in range(CCH):
                cmb = comb[(b, cc)]
                xvc = xv[(b, cc)]
                for ij in range(KK):
                    di, dj = ij // K, ij % K
                    w_ap = cmb[:, ij * O + oc * 128: ij * O + oc * 128 + 128]
                    first = (cc == 0 and ij == 0)
                    last = (cc == CCH - 1 and ij == KK - 1)
                    for rb in range(NRB):
                        rhs = xvc[:, rb * RB_ROWS + di: rb * RB_ROWS + di + RB_ROWS,
                                  dj: dj + W]
                        nc.tensor.matmul(ptiles[rb][:], w_ap, rhs,
                                         start=first, stop=last)
            for rb in range(NRB):
                st = ostg_pool.tile([128, NBLK], F32, tag="ostg",
                                    name=f"st{b}_{oc}_{rb}")
                nc.scalar.copy(st[:], ptiles[rb][:])
                nc.sync.dma_start(
                    out_d[b, oc * 128:(oc + 1) * 128,
                          rb * RB_ROWS:(rb + 1) * RB_ROWS, :],
                    st[:])

        # ---- emission: software-pipelined across samples ----
        def emit_pipeline():
            emit_loads(0)
            rbc0 = emit_routing(0)
            emit_mixing(0, rbc0)
            pend_rbc = {}
            for b in range(BL):
                if b + 1 < BL:
                    emit_loads(b + 1)
                emit_conv_ochunk(b, 0)
                if b + 1 < BL:
                    pend_rbc[b + 1] = emit_routing(b + 1)
                emit_conv_ochunk(b, 1)
                if b + 1 < BL:
                    emit_mixing(b + 1, pend_rbc[b + 1])

        if loop_n is not None:
            # on-device HW loop around the whole pipeline (for timing)
            with tc.For_i(0, loop_n, 1):
                emit_pipeline()
        else:
            for _rep in range(reps):
                emit_pipeline()

    nc.compile()
    return nc


def _prep_inputs(x, experts, rw1, rb1, rw2, rb2):
    x = np.asarray(x, dtype=np.float32)
    x = np.ascontiguousarray(
        np.pad(x, ((0, 0), (0, 0), (1, 1), (1, 1))))
    experts = np.asarray(experts, dtype=np.float32)
    # [E,O,C,K,K] -> [C,E,K*K,O]
    ex_t = np.ascontiguousarray(
        np.transpose(experts, (2, 0, 3, 4, 1)).reshape(C, E, KK, O))
    rw1t = np.ascontiguousarray(
        (np.asarray(rw1, dtype=np.float32) / float(H * W)).T)
    rb1v = np.ascontiguousarray(np.asarray(rb1, dtype=np.float32).reshape(HID, 1))
    rw2t = np.ascontiguousarray(np.asarray(rw2, dtype=np.float32).T)
    rb2v = np.ascontiguousarray(np.asarray(rb2, dtype=np.float32).reshape(1, E))
    in_maps = []
    for i in range(NCORES):
        in_maps.append({
            "x": np.ascontiguousarray(x[i * BL:(i + 1) * BL]),
            "experts_t": ex_t,
            "rw1t": rw1t,
            "rb1": rb1v,
            "rw2t": rw2t,
            "rb2": rb2v,
        })
    return in_maps


def run(inputs, trace=False, use_f32r=True, **trace_kwargs):
    """Build (cached), run on 8 cores, return (full_out, BassKernelResults)."""
    key = ("prog", use_f32r)
    if key not in _CACHE:
        _CACHE[key] = _build_program(use_f32r=use_f32r)
    nc = _CACHE[key]
    in_maps = _prep_inputs(**inputs)
    res = run_bass_kernel_spmd(nc, in_maps, list(range(NCORES)),
                               trace=trace, **trace_kwargs)
    out = np.concatenate([res.results[i]["out"] for i in range(NCORES)], axis=0)
    return out, res


def kernel(x, experts, rw1, rb1, rw2, rb2):
    out, _ = run(dict(x=x, experts=experts, rw1=rw1, rb1=rb1, rw2=rw2, rb2=rb2))
    return out


# revision 24
# speedup vs baseline: 3.6568x; 3.6568x over previous
"""CondConv2d (MoE routed conv) Trainium2 Bass kernel.

Strategy
--------
Data-parallel over batch B=32 across 8 NeuronCores (4 samples/core); the
expert bank + routing params are replicated.  Per core and sample:

  1. x[b] is DMA'd into SBUF as two zero-padded [128, 58*58] tiles
     (C=256 split across 2 partition chunks, H/W padded by 1).
  2. pooled = sum_hw(x)  (DVE free-dim reduce; pad zeros don't matter).
     Routing MLP (relu + softmax over E=4) runs on PE/ACT/DVE; the 4
     routing scalars are partition-broadcast via GPSIMD.
  3. Expert mixing: combined[c,(ij,o)] = sum_e r_e * experts[c,e,(ij,o)]
     as a fused scalar_tensor_tensor chain on DVE.  experts are
     host-relayout'ed to [C, E, 3*3, O] so the SBUF slabs DMA contiguously
     and mixed tiles are directly in matmul-lhsT orientation.
  4. Conv = 9 shifted 1x1 convs accumulated in PSUM: for each output
     chunk [128 o, 448] (8 rows x 56 cols), accumulate 2 (c-chunk) x 9
     (shift) matmuls, N=448, operands bitcast to float32r (full PE rate
     at fp32 storage).
  5. PSUM -> SBUF copies on ACT, then HWDGE DMA to HBM.
"""

import numpy as np
from contextlib import ExitStack

import concourse.bass as bass
import concourse.bacc as bacc
import concourse.mybir as mybir
import concourse.tile as tile
from concourse.bass_utils import run_bass_kernel_spmd

F32 = mybir.dt.float32
F32R = mybir.dt.float32r
AF = mybir.ActivationFunctionType
ALU = mybir.AluOpType
AX = mybir.AxisListType

# Problem shapes (hardcoded per contract).
B, C, H, W = 32, 256, 56, 56
E, O, K = 4, 256, 3
HID = 64
NCORES = 8
BL = B // NCORES          # samples per core
CCH = C // 128            # c partition chunks
OCH = O // 128            # o partition chunks
HP, WP = H + 2, W + 2     # padded
RB_ROWS = 8               # output rows per matmul
NRB = H // RB_ROWS        # 7 row blocks
NBLK = RB_ROWS * W        # 448 = matmul free size
KK = K * K

_CACHE = {}


def _build_program(use_f32r=True, reps=1, loop_n=None):
    nc = bacc.Bacc("TRN2", target_bir_lowering=False, debug=False)

    x_d = nc.dram_tensor("x", [BL, C, HP, WP], F32, kind="ExternalInput").ap()
    ex_d = nc.dram_tensor("experts_t", [C, E, KK, O], F32, kind="ExternalInput").ap()
    rw1t_d = nc.dram_tensor("rw1t", [C, HID], F32, kind="ExternalInput").ap()
    rb1_d = nc.dram_tensor("rb1", [HID, 1], F32, kind="ExternalInput").ap()
    rw2t_d = nc.dram_tensor("rw2t", [HID, E], F32, kind="ExternalInput").ap()
    rb2_d = nc.dram_tensor("rb2", [1, E], F32, kind="ExternalInput").ap()
    out_d = nc.dram_tensor("out", [BL, O, H, W], F32, kind="ExternalOutput").ap()

    mmdt = F32R if use_f32r else F32

    with tile.TileContext(nc) as tc, ExitStack() as ctx:
        const_pool = ctx.enter_context(tc.tile_pool(name="const", bufs=1))
        xpad_pool = ctx.enter_context(tc.tile_pool(name="xpad", bufs=2 * CCH))
        xstg_pool = ctx.enter_context(tc.tile_pool(name="xstg", bufs=2))
        comb_pool = ctx.enter_context(tc.tile_pool(name="comb", bufs=2 * CCH))
        scr_pool = ctx.enter_context(tc.tile_pool(name="scr", bufs=1))
        ostg_pool = ctx.enter_context(tc.tile_pool(name="ostg", bufs=4))
        small_pool = ctx.enter_context(tc.tile_pool(name="small", bufs=2))
        cpsum_pool = ctx.enter_context(tc.tile_pool(name="cpsum", bufs=NRB, space="PSUM"))
        mpsum_pool = ctx.enter_context(tc.tile_pool(name="mpsum", bufs=1, space="PSUM"))

        # ---- constants / parameters (preload once) ----
        rw1t_t = []
        for cc in range(CCH):
            t = const_pool.tile([128, HID], F32, name=f"rw1t{cc}")
            nc.sync.dma_start(t[:], rw1t_d[cc * 128:(cc + 1) * 128, :])
            rw1t_t.append(t)
        rb1_t = const_pool.tile([HID, 1], F32, name="rb1")
        nc.sync.dma_start(rb1_t[:], rb1_d[:])
        rw2t_t = const_pool.tile([HID, E], F32, name="rw2t")
        nc.sync.dma_start(rw2t_t[:], rw2t_d[:])
        rb2_t = const_pool.tile([1, E], F32, name="rb2")
        nc.sync.dma_start(rb2_t[:], rb2_d[:])
        ones_t = const_pool.tile([1, 128], F32, name="ones")
        nc.vector.memset(ones_t[:], 1.0)

        slabs = []
        for cc in range(CCH):
            t = const_pool.tile([128, E * KK * O], F32, name=f"slab{cc}")
            nc.sync.dma_start(t[:], ex_d[cc * 128:(cc + 1) * 128])
            slabs.append(t)

        # per-sample state
        xv = {}       # (b, cc) -> padded x tile viewed [128, HP, WP]
        comb = {}     # (b, cc) -> combined weights [128, KK*O]

        def emit_loads(b):
            for cc in range(CCH):
                if use_f32r:
                    # contiguous HWDGE DMA at f32 into staging, then a DVE
                    # rounding copy into the f32r tile the PE consumes
                    stg = xstg_pool.tile([128, HP * WP], F32, tag="xstg",
                                         name=f"xs{b}_{cc}")
                    nc.sync.dma_start(stg[:], x_d[b, cc * 128:(cc + 1) * 128])
                    t = xpad_pool.tile([128, HP * WP], F32R, tag="xpad",
                                       name=f"xp{b}_{cc}")
                    nc.vector.tensor_copy(t[:], stg[:])
                else:
                    t = xpad_pool.tile([128, HP * WP], F32, tag="xpad",
                                       name=f"xp{b}_{cc}")
                    nc.sync.dma_start(t[:], x_d[b, cc * 128:(cc + 1) * 128])
                xv[(b, cc)] = t.rearrange("p (h w) -> p h w", w=WP)

        def emit_routing(b):
            pooled = []
            for cc in range(CCH):
                p = small_pool.tile([128, 1], F32, tag="pooled", bufs=4,
                                    name=f"pool{b}_{cc}")
                nc.vector.reduce_sum(out=p[:], in_=xv[(b, cc)][:], axis=AX.XY)
                pooled.append(p)
            mps = mpsum_pool.tile([128, 512], F32, tag="mps", name=f"mps{b}")
            for cc in range(CCH):
                nc.tensor.matmul(mps[0:HID, 0:1], rw1t_t[cc][:], pooled[cc][:],
                                 start=(cc == 0), stop=(cc == CCH - 1))
            h_sb = small_pool.tile([HID, 1], F32, tag="h", name=f"h{b}")
            nc.scalar.activation(h_sb[:], mps[0:HID, 0:1], AF.Relu, bias=rb1_t[:])
            nc.tensor.matmul(mps[0:1, 4:4 + E], h_sb[:], rw2t_t[:],
                             start=True, stop=True)
            ze = small_pool.tile([1, E], F32, tag="ze", name=f"ze{b}")
            nc.vector.tensor_add(ze[:], mps[0:1, 4:4 + E], rb2_t[:])
            es = small_pool.tile([1, E], F32, tag="es", name=f"es{b}")
            nc.scalar.activation(es[:], ze[:], AF.Exp)
            ssum = small_pool.tile([1, 1], F32, tag="ssum", name=f"ss{b}")
            nc.vector.reduce_sum(out=ssum[:], in_=es[:], axis=AX.X)
            rec = small_pool.tile([1, 1], F32, tag="rec", name=f"rec{b}")
            nc.vector.reciprocal(rec[:], ssum[:])
            r_t = small_pool.tile([1, E], F32, tag="r", name=f"r{b}")
            nc.vector.tensor_scalar_mul(r_t[:], es[:], rec[:])
            # broadcast r across partitions: ones[1,128].T @ r[1,E] on PE
            nc.tensor.matmul(mps[0:128, 8:8 + E], ones_t[:], r_t[:],
                             start=True, stop=True)
            rbc = small_pool.tile([128, E], F32, tag="rbc", name=f"rbc{b}")
            nc.scalar.copy(rbc[:], mps[0:128, 8:8 + E])
            return rbc

        def emit_mixing(b, rbc):
            S = E * KK * O // CCH * CCH  # noqa (slab free size is E*KK*O)
            seg = KK * O
            for cc in range(CCH):
                slab = slabs[cc]
                a = scr_pool.tile([128, seg], F32, tag="scr", name=f"scr{b}_{cc}")
                nc.vector.tensor_scalar_mul(a[:], slab[:, 0:seg], rbc[:, 0:1])
                for e in range(1, E - 1):
                    nc.vector.scalar_tensor_tensor(
                        a[:], slab[:, e * seg:(e + 1) * seg], rbc[:, e:e + 1],
                        a[:], op0=ALU.mult, op1=ALU.add)
                cmb = comb_pool.tile([128, seg], mmdt, tag="comb",
                                     name=f"cmb{b}_{cc}")
                nc.vector.scalar_tensor_tensor(
                    cmb[:], slab[:, (E - 1) * seg:E * seg], rbc[:, E - 1:E],
                    a[:], op0=ALU.mult, op1=ALU.add)
                comb[(b, cc)] = cmb

        def emit_conv_ochunk(b, oc):
            ptiles = [cpsum_pool.tile([128, NBLK], F32, tag="cps",
                                      name=f"cp{b}_{oc}_{rb}")
                      for rb in range(NRB)]
            for cc in range(CCH):
                cmb = comb[(b, cc)]
                xvc = xv[(b, cc)]
                for ij in range(KK):
                    di, dj = ij // K, ij % K
                    w_ap = cmb[:, ij * O + oc * 128: ij * O + oc * 128 + 128]
                    first = (cc == 0 and ij == 0)
                    last = (cc == CCH - 1 and ij == KK - 1)
                    for rb in range(NRB):
                        rhs = xvc[:, rb * RB_ROWS + di: rb * RB_ROWS + di + RB_ROWS,
                                  dj: dj + W]
                        nc.tensor.matmul(ptiles[rb][:], w_ap, rhs,
                                         start=first, stop=last)
            for rb in range(NRB):
                st = ostg_pool.tile([128, NBLK], F32, tag="ostg",
                                    name=f"st{b}_{oc}_{rb}")
                nc.scalar.copy(st[:], ptiles[rb][:])
                nc.sync.dma_start(
                    out_d[b, oc * 128:(oc + 1) * 128,
                          rb * RB_ROWS:(rb + 1) * RB_ROWS, :],
                    st[:])

        # ---- emission: software-pipelined across samples ----
        def emit_pipeline():
            emit_loads(0)
            rbc0 = emit_routing(0)
            emit_mixing(0, rbc0)
            pend_rbc = {}
            for b in range(BL):
                if b + 1 < BL:
                    emit_loads(b + 1)
                emit_conv_ochunk(b, 0)
                if b + 1 < BL:
                    pend_rbc[b + 1] = emit_routing(b + 1)
                emit_conv_ochunk(b, 1)
                if b + 1 < BL:
                    emit_mixing(b + 1, pend_rbc[b + 1])

        if loop_n is not None:
            # on-device HW loop around the whole pipeline (for timing)
            with tc.For_i(0, loop_n, 1):
                emit_pipeline()
        else:
            for _rep in range(reps):
                emit_pipeline()

    nc.compile()
    return nc


def _prep_inputs(x, experts, rw1, rb1, rw2, rb2):
    x = np.asarray(x, dtype=np.float32)
    x = np.ascontiguousarray(
        np.pad(x, ((0, 0), (0, 0), (1, 1), (1, 1))))
    experts = np.asarray(experts, dtype=np.float32)
    # [E,O,C,K,K] -> [C,E,K*K,O]
    ex_t = np.ascontiguousarray(
        np.transpose(experts, (2, 0, 3, 4, 1)).reshape(C, E, KK, O))
    rw1t = np.ascontiguousarray(
        (np.asarray(rw1, dtype=np.float32) / float(H * W)).T)
    rb1v = np.ascontiguousarray(np.asarray(rb1, dtype=np.float32).reshape(HID, 1))
    rw2t = np.ascontiguousarray(np.asarray(rw2, dtype=np.float32).T)
    rb2v = np.ascontiguousarray(np.asarray(rb2, dtype=np.float32).reshape(1, E))
    in_maps = []
    for i in range(NCORES):
        in_maps.append({
            "x": np.ascontiguousarray(x[i * BL:(i + 1) * BL]),
            "experts_t": ex_t,
            "rw1t": rw1t,
            "rb1": rb1v,
            "rw2t": rw2t,
            "rb2": rb2v,
        })
    return in_maps


def run(inputs, trace=False, use_f32r=True, **trace_kwargs):
    """Build (cached), run on 8 cores, return (full_out, BassKernelResults)."""
    key = ("prog", use_f32r)
    if key not in _CACHE:
        _CACHE[key] = _build_program(use_f32r=use_f32r)
    nc = _CACHE[key]
    in_maps = _prep_inputs(**inputs)
    res = run_bass_kernel_spmd(nc, in_maps, list(range(NCORES)),
                               trace=trace, **trace_kwargs)
    out = np.concatenate([res.results[i]["out"] for i in range(NCORES)], axis=0)
    return out, res


def kernel(x, experts, rw1, rb1, rw2, rb2):
    out, _ = run(dict(x=x, experts=experts, rw1=rw1, rb1=rb1, rw2=rw2, rb2=rb2))
    return out


# revision 31
# speedup vs baseline: 4.0438x; 1.1058x over previous
"""CondConv2d (MoE routed conv) Trainium2 Bass kernel.

Strategy
--------
Data-parallel over batch B=32 across 8 NeuronCores (4 samples/core); the
expert bank + routing params are replicated.  Per core and sample:

  1. x[b] is DMA'd into SBUF as two zero-padded [128, 58*58] tiles
     (C=256 split across 2 partition chunks, H/W padded by 1).
  2. pooled = sum_hw(x)  (DVE free-dim reduce; pad zeros don't matter).
     Routing MLP (relu + softmax over E=4) runs on PE/ACT/DVE; the 4
     routing scalars are partition-broadcast via GPSIMD.
  3. Expert mixing: combined[c,(ij,o)] = sum_e r_e * experts[c,e,(ij,o)]
     as a fused scalar_tensor_tensor chain on DVE.  experts are
     host-relayout'ed to [C, E, 3*3, O] so the SBUF slabs DMA contiguously
     and mixed tiles are directly in matmul-lhsT orientation.
  4. Conv = 9 shifted 1x1 convs accumulated in PSUM: for each output
     chunk [128 o, 448] (8 rows x 56 cols), accumulate 2 (c-chunk) x 9
     (shift) matmuls, N=448, operands bitcast to float32r (full PE rate
     at fp32 storage).
  5. PSUM -> SBUF copies on ACT, then HWDGE DMA to HBM.
"""

import numpy as np
from contextlib import ExitStack

import concourse.bass as bass
import concourse.bacc as bacc
import concourse.mybir as mybir
import concourse.tile as tile
from concourse.bass_utils import run_bass_kernel_spmd

F32 = mybir.dt.float32
F32R = mybir.dt.float32r
AF = mybir.ActivationFunctionType
ALU = mybir.AluOpType
AX = mybir.AxisListType

# Problem shapes (hardcoded per contract).
B, C, H, W = 32, 256, 56, 56
E, O, K = 4, 256, 3
HID = 64
NCORES = 8
BL = B // NCORES          # samples per core
CCH = C // 128            # c partition chunks
OCH = O // 128            # o partition chunks
HP, WP = H + 2, W + 2     # padded
RB_ROWS = 8               # output rows per matmul
NRB = H // RB_ROWS        # 7 row blocks
NBLK = RB_ROWS * W        # 448 = matmul free size
KK = K * K

_CACHE = {}


def _build_program(use_f32r=True, reps=1, loop_n=None):
    nc = bacc.Bacc("TRN2", target_bir_lowering=False, debug=False)

    x_d = nc.dram_tensor("x", [BL, C, HP, WP], F32, kind="ExternalInput").ap()
    ex_d = nc.dram_tensor("experts_t", [C, E, KK, O], F32, kind="ExternalInput").ap()
    rw1t_d = nc.dram_tensor("rw1t", [C, HID], F32, kind="ExternalInput").ap()
    rb1_d = nc.dram_tensor("rb1", [HID, 1], F32, kind="ExternalInput").ap()
    rw2t_d = nc.dram_tensor("rw2t", [HID, E], F32, kind="ExternalInput").ap()
    rb2_d = nc.dram_tensor("rb2", [1, E], F32, kind="ExternalInput").ap()
    out_d = nc.dram_tensor("out", [BL, O, H, W], F32, kind="ExternalOutput").ap()

    mmdt = F32R if use_f32r else F32

    with tile.TileContext(nc) as tc, ExitStack() as ctx:
        const_pool = ctx.enter_context(tc.tile_pool(name="const", bufs=1))
        xpad_pool = ctx.enter_context(tc.tile_pool(name="xpad", bufs=2 * CCH))
        xstg_pool = ctx.enter_context(tc.tile_pool(name="xstg", bufs=2))
        comb_pool = ctx.enter_context(tc.tile_pool(name="comb", bufs=2 * CCH))
        scr_pool = ctx.enter_context(tc.tile_pool(name="scr", bufs=1))
        ostg_pool = ctx.enter_context(tc.tile_pool(name="ostg", bufs=4))
        small_pool = ctx.enter_context(tc.tile_pool(name="small", bufs=2))
        cpsum_pool = ctx.enter_context(tc.tile_pool(name="cpsum", bufs=NRB, space="PSUM"))
        mpsum_pool = ctx.enter_context(tc.tile_pool(name="mpsum", bufs=1, space="PSUM"))

        # ---- constants / parameters (preload once) ----
        rw1t_t = []
        for cc in range(CCH):
            t = const_pool.tile([128, HID], F32, name=f"rw1t{cc}")
            nc.sync.dma_start(t[:], rw1t_d[cc * 128:(cc + 1) * 128, :])
            rw1t_t.append(t)
        rb1_t = const_pool.tile([HID, 1], F32, name="rb1")
        nc.sync.dma_start(rb1_t[:], rb1_d[:])
        rw2t_t = const_pool.tile([HID, E], F32, name="rw2t")
        nc.sync.dma_start(rw2t_t[:], rw2t_d[:])
        rb2_t = const_pool.tile([1, E], F32, name="rb2")
        nc.sync.dma_start(rb2_t[:], rb2_d[:])
        ones_t = const_pool.tile([1, 128], F32, name="ones")
        nc.vector.memset(ones_t[:], 1.0)

        slabs = []

        def emit_slab_loads():
            slabs.clear()
            for cc in range(CCH):
                t = const_pool.tile([128, E * KK * O], F32, name=f"slab{cc}")
                nc.sync.dma_start(t[:], ex_d[cc * 128:(cc + 1) * 128])
                slabs.append(t)

        # per-sample state
        xv = {}       # (b, cc) -> padded x tile viewed [128, HP, WP]
        comb = {}     # (b, cc) -> combined weights [128, KK*O]
        pooled_t = {}  # (b, cc) -> [128, 1] sum over h*w

        def emit_loads(b):
            for cc in range(CCH):
                p = small_pool.tile([128, 1], F32, tag="pooled", bufs=4,
                                    name=f"pool{b}_{cc}")
                if use_f32r:
                    # contiguous HWDGE DMA at f32 into staging, then a DVE
                    # rounding pass (f32 -> f32r) that also produces the
                    # h*w pooled sum via accum_out
                    stg = xstg_pool.tile([128, HP * WP], F32, tag="xstg",
                                         name=f"xs{b}_{cc}")
                    nc.sync.dma_start(stg[:], x_d[b, cc * 128:(cc + 1) * 128])
                    t = xpad_pool.tile([128, HP * WP], F32R, tag="xpad",
                                       name=f"xp{b}_{cc}")
                    nc.vector.tensor_scalar(
                        t[:], stg[:], 1.0, None, op0=ALU.mult, op1=ALU.add,
                        accum_out=p[:])
                else:
                    t = xpad_pool.tile([128, HP * WP], F32, tag="xpad",
                                       name=f"xp{b}_{cc}")
                    nc.sync.dma_start(t[:], x_d[b, cc * 128:(cc + 1) * 128])
                    nc.vector.reduce_sum(out=p[:], in_=t[:], axis=AX.XY)
                xv[(b, cc)] = t.rearrange("p (h w) -> p h w", w=WP)
                pooled_t[(b, cc)] = p

        def emit_routing(b):
            mps = mpsum_pool.tile([128, 512], F32, tag="mps", name=f"mps{b}")
            for cc in range(CCH):
                nc.tensor.matmul(mps[0:HID, 0:1], rw1t_t[cc][:],
                                 pooled_t[(b, cc)][:],
                                 start=(cc == 0), stop=(cc == CCH - 1))
            h_sb = small_pool.tile([HID, 1], F32, tag="h", name=f"h{b}")
            nc.scalar.activation(h_sb[:], mps[0:HID, 0:1], AF.Relu, bias=rb1_t[:])
            nc.tensor.matmul(mps[0:1, 4:4 + E], h_sb[:], rw2t_t[:],
                             start=True, stop=True)
            ze = small_pool.tile([1, E], F32, tag="ze", name=f"ze{b}")
            nc.vector.tensor_add(ze[:], mps[0:1, 4:4 + E], rb2_t[:])
            es = small_pool.tile([1, E], F32, tag="es", name=f"es{b}")
            nc.scalar.activation(es[:], ze[:], AF.Exp)
            ssum = small_pool.tile([1, 1], F32, tag="ssum", name=f"ss{b}")
            nc.vector.reduce_sum(out=ssum[:], in_=es[:], axis=AX.X)
            rec = small_pool.tile([1, 1], F32, tag="rec", name=f"rec{b}")
            nc.vector.reciprocal(rec[:], ssum[:])
            r_t = small_pool.tile([1, E], F32, tag="r", name=f"r{b}")
            nc.vector.tensor_scalar_mul(r_t[:], es[:], rec[:])
            # broadcast r across partitions: ones[1,128].T @ r[1,E] on PE
            nc.tensor.matmul(mps[0:128, 8:8 + E], ones_t[:], r_t[:],
                             start=True, stop=True)
            rbc = small_pool.tile([128, E], F32, tag="rbc", name=f"rbc{b}")
            nc.scalar.copy(rbc[:], mps[0:128, 8:8 + E])
            return rbc

        def emit_mixing(b, rbc):
            seg = KK * O          # per-expert block within a slab
            NG = 3                # ij-groups; PE can start after group 0
            gsz = seg // NG
            for cc in range(CCH):
                slab = slabs[cc]
                cmb = comb_pool.tile([128, seg], mmdt, tag="comb",
                                     name=f"cmb{b}_{cc}")
                for g in range(NG):
                    lo = g * gsz
                    a = scr_pool.tile([128, gsz], F32, tag="scr",
                                      name=f"scr{b}_{cc}_{g}")
                    nc.vector.tensor_scalar_mul(
                        a[:], slab[:, lo:lo + gsz], rbc[:, 0:1])
                    for e in range(1, E - 1):
                        nc.vector.scalar_tensor_tensor(
                            a[:], slab[:, e * seg + lo:e * seg + lo + gsz],
                            rbc[:, e:e + 1], a[:], op0=ALU.mult, op1=ALU.add)
                    nc.vector.scalar_tensor_tensor(
                        cmb[:, lo:lo + gsz],
                        slab[:, (E - 1) * seg + lo:(E - 1) * seg + lo + gsz],
                        rbc[:, E - 1:E], a[:], op0=ALU.mult, op1=ALU.add)
                comb[(b, cc)] = cmb

        def emit_conv_ochunk(b, oc):
            ptiles = [cpsum_pool.tile([128, NBLK], F32, tag="cps",
                                      name=f"cp{b}_{oc}_{rb}")
                      for rb in range(NRB)]
            for cc in range(CCH):
                cmb = comb[(b, cc)]
                xvc = xv[(b, cc)]
                for ij in range(KK):
                    di, dj = ij // K, ij % K
                    w_ap = cmb[:, ij * O + oc * 128: ij * O + oc * 128 + 128]
                    first = (cc == 0 and ij == 0)
                    last = (cc == CCH - 1 and ij == KK - 1)
                    for rb in range(NRB):
                        rhs = xvc[:, rb * RB_ROWS + di: rb * RB_ROWS + di + RB_ROWS,
                                  dj: dj + W]
                        nc.tensor.matmul(ptiles[rb][:], w_ap, rhs,
                                         start=first, stop=last)
            for rb in range(NRB):
                st = ostg_pool.tile([128, NBLK], F32, tag="ostg",
                                    name=f"st{b}_{oc}_{rb}")
                nc.scalar.copy(st[:], ptiles[rb][:])
                nc.sync.dma_start(
                    out_d[b, oc * 128:(oc + 1) * 128,
                          rb * RB_ROWS:(rb + 1) * RB_ROWS, :],
                    st[:])

        # ---- emission: software-pipelined across samples ----
        def emit_pipeline():
            emit_loads(0)
            emit_slab_loads()
            rbc0 = emit_routing(0)
            emit_mixing(0, rbc0)
            pend_rbc = {}
            for b in range(BL):
                if b + 1 < BL:
                    emit_loads(b + 1)
                emit_conv_ochunk(b, 0)
                if b + 1 < BL:
                    pend_rbc[b + 1] = emit_routing(b + 1)
                emit_conv_ochunk(b, 1)
                if b + 1 < BL:
                    emit_mixing(b + 1, pend_rbc[b + 1])

        if loop_n is not None:
            # on-device HW loop around the whole pipeline (for timing)
            with tc.For_i(0, loop_n, 1):
                emit_pipeline()
        else:
            for _rep in range(reps):
                emit_pipeline()

    nc.compile()
    return nc


def _prep_inputs(x, experts, rw1, rb1, rw2, rb2):
    x = np.asarray(x, dtype=np.float32)
    x = np.ascontiguousarray(
        np.pad(x, ((0, 0), (0, 0), (1, 1), (1, 1))))
    experts = np.asarray(experts, dtype=np.float32)
    # [E,O,C,K,K] -> [C,E,K*K,O]
    ex_t = np.ascontiguousarray(
        np.transpose(experts, (2, 0, 3, 4, 1)).reshape(C, E, KK, O))
    rw1t = np.ascontiguousarray(
        (np.asarray(rw1, dtype=np.float32) / float(H * W)).T)
    rb1v = np.ascontiguousarray(np.asarray(rb1, dtype=np.float32).reshape(HID, 1))
    rw2t = np.ascontiguousarray(np.asarray(rw2, dtype=np.float32).T)
    rb2v = np.ascontiguousarray(np.asarray(rb2, dtype=np.float32).reshape(1, E))
    in_maps = []
    for i in range(NCORES):
        in_maps.append({
            "x": np.ascontiguousarray(x[i * BL:(i + 1) * BL]),
            "experts_t": ex_t,
            "rw1t": rw1t,
            "rb1": rb1v,
            "rw2t": rw2t,
            "rb2": rb2v,
        })
    return in_maps


def run(inputs, trace=False, use_f32r=True, **trace_kwargs):
    """Build (cached), run on 8 cores, return (full_out, BassKernelResults)."""
    key = ("prog", use_f32r)
    if key not in _CACHE:
        _CACHE[key] = _build_program(use_f32r=use_f32r)
    nc = _CACHE[key]
    in_maps = _prep_inputs(**inputs)
    res = run_bass_kernel_spmd(nc, in_maps, list(range(NCORES)),
                               trace=trace, **trace_kwargs)
    out = np.concatenate([res.results[i]["out"] for i in range(NCORES)], axis=0)
    return out, res


def kernel(x, experts, rw1, rb1, rw2, rb2):
    out, _ = run(dict(x=x, experts=experts, rw1=rw1, rb1=rb1, rw2=rw2, rb2=rb2))
    return out


# revision 34
# speedup vs baseline: 4.4488x; 1.1002x over previous
"""CondConv2d (MoE routed conv) Trainium2 Bass kernel.

Strategy
--------
Data-parallel over batch B=32 across 8 NeuronCores (4 samples/core); the
expert bank + routing params are replicated.  Per core and sample:

  1. x[b] is DMA'd into SBUF as two zero-padded [128, 58*58] tiles
     (C=256 split across 2 partition chunks, H/W padded by 1).
  2. pooled = sum_hw(x)  (DVE free-dim reduce; pad zeros don't matter).
     Routing MLP (relu + softmax over E=4) runs on PE/ACT/DVE; the 4
     routing scalars are partition-broadcast via GPSIMD.
  3. Expert mixing: combined[c,(ij,o)] = sum_e r_e * experts[c,e,(ij,o)]
     as a fused scalar_tensor_tensor chain on DVE.  experts are
     host-relayout'ed to [C, E, 3*3, O] so the SBUF slabs DMA contiguously
     and mixed tiles are directly in matmul-lhsT orientation.
  4. Conv = 9 shifted 1x1 convs accumulated in PSUM: for each output
     chunk [128 o, 448] (8 rows x 56 cols), accumulate 2 (c-chunk) x 9
     (shift) matmuls, N=448, operands bitcast to float32r (full PE rate
     at fp32 storage).
  5. PSUM -> SBUF copies on ACT, then HWDGE DMA to HBM.
"""

import numpy as np
from contextlib import ExitStack

import concourse.bass as bass
import concourse.bacc as bacc
import concourse.mybir as mybir
import concourse.tile as tile
from concourse.bass_utils import run_bass_kernel_spmd

F32 = mybir.dt.float32
F32R = mybir.dt.float32r
AF = mybir.ActivationFunctionType
ALU = mybir.AluOpType
AX = mybir.AxisListType

# Problem shapes (hardcoded per contract).
B, C, H, W = 32, 256, 56, 56
E, O, K = 4, 256, 3
HID = 64
NCORES = 8
BL = B // NCORES          # samples per core
CCH = C // 128            # c partition chunks
OCH = O // 128            # o partition chunks
HP, WP = H + 2, W + 2     # padded
RB_ROWS = 8               # output rows per matmul
NRB = H // RB_ROWS        # 7 row blocks
NBLK = RB_ROWS * W        # 448 = matmul free size
KK = K * K

_CACHE = {}


def _build_program(use_f32r=True, reps=1, loop_n=None):
    nc = bacc.Bacc("TRN2", target_bir_lowering=False, debug=False)

    x_d = nc.dram_tensor("x", [BL, C, HP, WP], F32, kind="ExternalInput").ap()
    ex_d = nc.dram_tensor("experts_t", [C, E, KK, O], F32, kind="ExternalInput").ap()
    # packed routing params: [:,0:64]=rw1t cc0, [:,64:128]=rw1t cc1,
    # [0:64,128]=rb1, [0:64,129:133]=rw2t, [0:1,133:137]=rb2
    RP = 2 * HID + 1 + E + E
    rp_d = nc.dram_tensor("rparams", [128, RP], F32, kind="ExternalInput").ap()
    out_d = nc.dram_tensor("out", [BL, O, H, W], F32, kind="ExternalOutput").ap()

    mmdt = F32R if use_f32r else F32

    with tile.TileContext(nc) as tc, ExitStack() as ctx:
        const_pool = ctx.enter_context(tc.tile_pool(name="const", bufs=1))
        xpad_pool = ctx.enter_context(tc.tile_pool(name="xpad", bufs=2 * CCH))
        xstg_pool = ctx.enter_context(tc.tile_pool(name="xstg", bufs=2))
        comb_pool = ctx.enter_context(tc.tile_pool(name="comb", bufs=2 * CCH))
        scr_pool = ctx.enter_context(tc.tile_pool(name="scr", bufs=1))
        ostg_pool = ctx.enter_context(tc.tile_pool(name="ostg", bufs=4))
        small_pool = ctx.enter_context(tc.tile_pool(name="small", bufs=2))
        cpsum_pool = ctx.enter_context(tc.tile_pool(name="cpsum", bufs=NRB, space="PSUM"))
        mpsum_pool = ctx.enter_context(tc.tile_pool(name="mpsum", bufs=1, space="PSUM"))

        # ---- constants / parameters (preload once, single DMA) ----
        rp_t = const_pool.tile([128, RP], F32, name="rp")
        nc.sync.dma_start(rp_t[:], rp_d[:])
        rw1t_t = [rp_t[:, 0:HID], rp_t[:, HID:2 * HID]]
        rb1_t = rp_t[0:HID, 2 * HID:2 * HID + 1]
        rw2t_t = rp_t[0:HID, 2 * HID + 1:2 * HID + 1 + E]
        rb2_t = rp_t[0:1, 2 * HID + 1 + E:2 * HID + 1 + 2 * E]
        ones_t = const_pool.tile([1, 128], F32, name="ones")
        nc.vector.memset(ones_t[:], 1.0)

        slabs = []

        def emit_slab_loads():
            slabs.clear()
            for cc in range(CCH):
                t = const_pool.tile([128, E * KK * O], F32, name=f"slab{cc}")
                nc.sync.dma_start(t[:], ex_d[cc * 128:(cc + 1) * 128])
                slabs.append(t)

        # per-sample state
        xv = {}       # (b, cc) -> padded x tile viewed [128, HP, WP]
        comb = {}     # (b, cc) -> combined weights [128, KK*O]
        pooled_t = {}  # (b, cc) -> [128, 1] sum over h*w

        def emit_loads(b):
            for cc in range(CCH):
                p = small_pool.tile([128, 1], F32, tag="pooled", bufs=4,
                                    name=f"pool{b}_{cc}")
                if use_f32r:
                    # contiguous HWDGE DMA at f32 into staging, then a DVE
                    # rounding pass (f32 -> f32r) that also produces the
                    # h*w pooled sum via accum_out
                    stg = xstg_pool.tile([128, HP * WP], F32, tag="xstg",
                                         name=f"xs{b}_{cc}")
                    nc.sync.dma_start(stg[:], x_d[b, cc * 128:(cc + 1) * 128])
                    t = xpad_pool.tile([128, HP * WP], F32R, tag="xpad",
                                       name=f"xp{b}_{cc}")
                    nc.vector.tensor_scalar(
                        t[:], stg[:], 1.0, None, op0=ALU.mult, op1=ALU.add,
                        accum_out=p[:])
                else:
                    t = xpad_pool.tile([128, HP * WP], F32, tag="xpad",
                                       name=f"xp{b}_{cc}")
                    nc.sync.dma_start(t[:], x_d[b, cc * 128:(cc + 1) * 128])
                    nc.vector.reduce_sum(out=p[:], in_=t[:], axis=AX.XY)
                xv[(b, cc)] = t.rearrange("p (h w) -> p h w", w=WP)
                pooled_t[(b, cc)] = p

        def emit_routing(b):
            mps = mpsum_pool.tile([128, 512], F32, tag="mps", name=f"mps{b}")
            for cc in range(CCH):
                nc.tensor.matmul(mps[0:HID, 0:1], rw1t_t[cc][:],
                                 pooled_t[(b, cc)][:],
                                 start=(cc == 0), stop=(cc == CCH - 1))
            h_sb = small_pool.tile([HID, 1], F32, tag="h", name=f"h{b}")
            nc.scalar.activation(h_sb[:], mps[0:HID, 0:1], AF.Relu, bias=rb1_t[:])
            nc.tensor.matmul(mps[0:1, 4:4 + E], h_sb[:], rw2t_t[:],
                             start=True, stop=True)
            ze = small_pool.tile([1, E], F32, tag="ze", name=f"ze{b}")
            nc.vector.tensor_add(ze[:], mps[0:1, 4:4 + E], rb2_t[:])
            es = small_pool.tile([1, E], F32, tag="es", name=f"es{b}")
            nc.scalar.activation(es[:], ze[:], AF.Exp)
            ssum = small_pool.tile([1, 1], F32, tag="ssum", name=f"ss{b}")
            nc.vector.reduce_sum(out=ssum[:], in_=es[:], axis=AX.X)
            rec = small_pool.tile([1, 1], F32, tag="rec", name=f"rec{b}")
            nc.vector.reciprocal(rec[:], ssum[:])
            r_t = small_pool.tile([1, E], F32, tag="r", name=f"r{b}")
            nc.vector.tensor_scalar_mul(r_t[:], es[:], rec[:])
            # broadcast r across partitions: ones[1,128].T @ r[1,E] on PE
            nc.tensor.matmul(mps[0:128, 8:8 + E], ones_t[:], r_t[:],
                             start=True, stop=True)
            rbc = small_pool.tile([128, E], F32, tag="rbc", name=f"rbc{b}")
            nc.scalar.copy(rbc[:], mps[0:128, 8:8 + E])
            return rbc

        def emit_mixing(b, rbc):
            seg = KK * O          # per-expert block within a slab
            NG = 3                # ij-groups; PE can start after group 0
            gsz = seg // NG
            for cc in range(CCH):
                slab = slabs[cc]
                cmb = comb_pool.tile([128, seg], mmdt, tag="comb",
                                     name=f"cmb{b}_{cc}")
                for g in range(NG):
                    lo = g * gsz
                    a = scr_pool.tile([128, gsz], F32, tag="scr",
                                      name=f"scr{b}_{cc}_{g}")
                    nc.vector.tensor_scalar_mul(
                        a[:], slab[:, lo:lo + gsz], rbc[:, 0:1])
                    for e in range(1, E - 1):
                        nc.vector.scalar_tensor_tensor(
                            a[:], slab[:, e * seg + lo:e * seg + lo + gsz],
                            rbc[:, e:e + 1], a[:], op0=ALU.mult, op1=ALU.add)
                    nc.vector.scalar_tensor_tensor(
                        cmb[:, lo:lo + gsz],
                        slab[:, (E - 1) * seg + lo:(E - 1) * seg + lo + gsz],
                        rbc[:, E - 1:E], a[:], op0=ALU.mult, op1=ALU.add)
                comb[(b, cc)] = cmb

        def emit_conv_ochunk(b, oc):
            ptiles = [cpsum_pool.tile([128, NBLK], F32, tag="cps",
                                      name=f"cp{b}_{oc}_{rb}")
                      for rb in range(NRB)]
            for cc in range(CCH):
                cmb = comb[(b, cc)]
                xvc = xv[(b, cc)]
                for ij in range(KK):
                    di, dj = ij // K, ij % K
                    w_ap = cmb[:, ij * O + oc * 128: ij * O + oc * 128 + 128]
                    first = (cc == 0 and ij == 0)
                    last = (cc == CCH - 1 and ij == KK - 1)
                    for rb in range(NRB):
                        rhs = xvc[:, rb * RB_ROWS + di: rb * RB_ROWS + di + RB_ROWS,
                                  dj: dj + W]
                        nc.tensor.matmul(ptiles[rb][:], w_ap, rhs,
                                         start=first, stop=last)
            for rb in range(NRB):
                st = ostg_pool.tile([128, NBLK], F32, tag="ostg",
                                    name=f"st{b}_{oc}_{rb}")
                nc.scalar.copy(st[:], ptiles[rb][:])
                nc.sync.dma_start(
                    out_d[b, oc * 128:(oc + 1) * 128,
                          rb * RB_ROWS:(rb + 1) * RB_ROWS, :],
                    st[:])

        # ---- emission: software-pipelined across samples ----
        def emit_pipeline():
            emit_loads(0)
            emit_slab_loads()
            rbc0 = emit_routing(0)
            emit_mixing(0, rbc0)
            pend_rbc = {}
            for b in range(BL):
                if b + 1 < BL:
                    emit_loads(b + 1)
                emit_conv_ochunk(b, 0)
                if b + 1 < BL:
                    pend_rbc[b + 1] = emit_routing(b + 1)
                emit_conv_ochunk(b, 1)
                if b + 1 < BL:
                    emit_mixing(b + 1, pend_rbc[b + 1])

        if loop_n is not None:
            # on-device HW loop around the whole pipeline (for timing)
            with tc.For_i(0, loop_n, 1):
                emit_pipeline()
        else:
            for _rep in range(reps):
                emit_pipeline()

    nc.compile()
    return nc


def _prep_inputs(x, experts, rw1, rb1, rw2, rb2):
    x = np.asarray(x, dtype=np.float32)
    x = np.ascontiguousarray(
        np.pad(x, ((0, 0), (0, 0), (1, 1), (1, 1))))
    experts = np.asarray(experts, dtype=np.float32)
    # [E,O,C,K,K] -> [C,E,K*K,O]
    ex_t = np.ascontiguousarray(
        np.transpose(experts, (2, 0, 3, 4, 1)).reshape(C, E, KK, O))
    rw1t = (np.asarray(rw1, dtype=np.float32) / float(H * W)).T  # [C, HID]
    rb1v = np.asarray(rb1, dtype=np.float32)
    rw2t = np.asarray(rw2, dtype=np.float32).T                   # [HID, E]
    rb2v = np.asarray(rb2, dtype=np.float32)
    RP = 2 * HID + 1 + 2 * E
    rp = np.zeros((128, RP), np.float32)
    rp[:, 0:HID] = rw1t[0:128]
    rp[:, HID:2 * HID] = rw1t[128:256]
    rp[0:HID, 2 * HID] = rb1v
    rp[0:HID, 2 * HID + 1:2 * HID + 1 + E] = rw2t
    rp[0, 2 * HID + 1 + E:2 * HID + 1 + 2 * E] = rb2v
    in_maps = []
    for i in range(NCORES):
        in_maps.append({
            "x": np.ascontiguousarray(x[i * BL:(i + 1) * BL]),
            "experts_t": ex_t,
            "rparams": rp,
        })
    return in_maps


def run(inputs, trace=False, use_f32r=True, **trace_kwargs):
    """Build (cached), run on 8 cores, return (full_out, BassKernelResults)."""
    key = ("prog", use_f32r)
    if key not in _CACHE:
        _CACHE[key] = _build_program(use_f32r=use_f32r)
    nc = _CACHE[key]
    in_maps = _prep_inputs(**inputs)
    res = run_bass_kernel_spmd(nc, in_maps, list(range(NCORES)),
                               trace=trace, **trace_kwargs)
    out = np.concatenate([res.results[i]["out"] for i in range(NCORES)], axis=0)
    return out, res


def kernel(x, experts, rw1, rb1, rw2, rb2):
    out, _ = run(dict(x=x, experts=experts, rw1=rw1, rb1=rb1, rw2=rw2, rb2=rb2))
    return out


# revision 36
# speedup vs baseline: 4.5580x; 1.0245x over previous
"""CondConv2d (MoE routed conv) Trainium2 Bass kernel.

Strategy
--------
Data-parallel over batch B=32 across 8 NeuronCores (4 samples/core); the
expert bank + routing params are replicated.  Per core and sample:

  1. x[b] is DMA'd into SBUF as two zero-padded [128, 58*58] tiles
     (C=256 split across 2 partition chunks, H/W padded by 1).
  2. pooled = sum_hw(x)  (DVE free-dim reduce; pad zeros don't matter).
     Routing MLP (relu + softmax over E=4) runs on PE/ACT/DVE; the 4
     routing scalars are partition-broadcast via GPSIMD.
  3. Expert mixing: combined[c,(ij,o)] = sum_e r_e * experts[c,e,(ij,o)]
     as a fused scalar_tensor_tensor chain on DVE.  experts are
     host-relayout'ed to [C, E, 3*3, O] so the SBUF slabs DMA contiguously
     and mixed tiles are directly in matmul-lhsT orientation.
  4. Conv = 9 shifted 1x1 convs accumulated in PSUM: for each output
     chunk [128 o, 448] (8 rows x 56 cols), accumulate 2 (c-chunk) x 9
     (shift) matmuls, N=448, operands bitcast to float32r (full PE rate
     at fp32 storage).
  5. PSUM -> SBUF copies on ACT, then HWDGE DMA to HBM.
"""

import numpy as np
from contextlib import ExitStack

import concourse.bass as bass
import concourse.bacc as bacc
import concourse.mybir as mybir
import concourse.tile as tile
from concourse.bass_utils import run_bass_kernel_spmd

F32 = mybir.dt.float32
F32R = mybir.dt.float32r
AF = mybir.ActivationFunctionType
ALU = mybir.AluOpType
AX = mybir.AxisListType

# Problem shapes (hardcoded per contract).
B, C, H, W = 32, 256, 56, 56
E, O, K = 4, 256, 3
HID = 64
NCORES = 8
BL = B // NCORES          # samples per core
CCH = C // 128            # c partition chunks
OCH = O // 128            # o partition chunks
HP, WP = H + 2, W + 2     # padded
RB_ROWS = 8               # output rows per matmul
NRB = H // RB_ROWS        # 7 row blocks
NBLK = RB_ROWS * W        # 448 = matmul free size
KK = K * K

_CACHE = {}


def _build_program(use_f32r=True, reps=1, loop_n=None):
    nc = bacc.Bacc("TRN2", target_bir_lowering=False, debug=False)

    x_d = nc.dram_tensor("x", [BL, C, HP, WP], F32, kind="ExternalInput").ap()
    ex_d = nc.dram_tensor("experts_t", [C, E, KK, O], F32, kind="ExternalInput").ap()
    # packed routing params: [:,0:64]=rw1t cc0, [:,64:128]=rw1t cc1,
    # [0:64,128]=rb1, [0:64,129:133]=rw2t, [0:1,133:137]=rb2
    RP = 2 * HID + 1 + E + E
    rp_d = nc.dram_tensor("rparams", [128, RP], F32, kind="ExternalInput").ap()
    out_d = nc.dram_tensor("out", [BL, O, H, W], F32, kind="ExternalOutput").ap()

    mmdt = F32R if use_f32r else F32

    with tile.TileContext(nc) as tc, ExitStack() as ctx:
        const_pool = ctx.enter_context(tc.tile_pool(name="const", bufs=1))
        xpad_pool = ctx.enter_context(tc.tile_pool(name="xpad", bufs=2 * CCH))
        xstg_pool = ctx.enter_context(tc.tile_pool(name="xstg", bufs=2))
        comb_pool = ctx.enter_context(tc.tile_pool(name="comb", bufs=2 * CCH))
        scr_pool = ctx.enter_context(tc.tile_pool(name="scr", bufs=1))
        ostg_pool = ctx.enter_context(tc.tile_pool(name="ostg", bufs=4))
        small_pool = ctx.enter_context(tc.tile_pool(name="small", bufs=2))
        cpsum_pool = ctx.enter_context(tc.tile_pool(name="cpsum", bufs=NRB, space="PSUM"))
        mpsum_pool = ctx.enter_context(tc.tile_pool(name="mpsum", bufs=1, space="PSUM"))

        # ---- constants / parameters (preload once, single DMA) ----
        rp_t = const_pool.tile([128, RP], F32, name="rp")
        nc.sync.dma_start(rp_t[:], rp_d[:])
        rw1t_t = [rp_t[:, 0:HID], rp_t[:, HID:2 * HID]]
        rb1_t = rp_t[0:HID, 2 * HID:2 * HID + 1]
        rw2t_t = rp_t[0:HID, 2 * HID + 1:2 * HID + 1 + E]
        rb2_t = rp_t[0:1, 2 * HID + 1 + E:2 * HID + 1 + 2 * E]
        ones_t = const_pool.tile([1, 128], F32, name="ones")
        nc.vector.memset(ones_t[:], 1.0)

        slabs = []   # [cc][e] -> [128, KK*O] tile

        def emit_slab_loads():
            slabs.clear()
            for cc in range(CCH):
                per_e = []
                for e in range(E):
                    t = const_pool.tile([128, KK * O], F32,
                                        name=f"slab{cc}e{e}")
                    nc.sync.dma_start(t[:], ex_d[cc * 128:(cc + 1) * 128, e])
                    per_e.append(t)
                slabs.append(per_e)

        # per-sample state
        xv = {}       # (b, cc) -> padded x tile viewed [128, HP, WP]
        comb = {}     # (b, cc) -> combined weights [128, KK*O]
        pooled_t = {}  # (b, cc) -> [128, 1] sum over h*w

        def emit_loads(b):
            for cc in range(CCH):
                p = small_pool.tile([128, 1], F32, tag="pooled", bufs=4,
                                    name=f"pool{b}_{cc}")
                if use_f32r:
                    # contiguous HWDGE DMA at f32 into staging, then a DVE
                    # rounding pass (f32 -> f32r) that also produces the
                    # h*w pooled sum via accum_out
                    stg = xstg_pool.tile([128, HP * WP], F32, tag="xstg",
                                         name=f"xs{b}_{cc}")
                    nc.sync.dma_start(stg[:], x_d[b, cc * 128:(cc + 1) * 128])
                    t = xpad_pool.tile([128, HP * WP], F32R, tag="xpad",
                                       name=f"xp{b}_{cc}")
                    nc.vector.tensor_scalar(
                        t[:], stg[:], 1.0, None, op0=ALU.mult, op1=ALU.add,
                        accum_out=p[:])
                else:
                    t = xpad_pool.tile([128, HP * WP], F32, tag="xpad",
                                       name=f"xp{b}_{cc}")
                    nc.sync.dma_start(t[:], x_d[b, cc * 128:(cc + 1) * 128])
                    nc.vector.reduce_sum(out=p[:], in_=t[:], axis=AX.XY)
                xv[(b, cc)] = t.rearrange("p (h w) -> p h w", w=WP)
                pooled_t[(b, cc)] = p

        def emit_routing(b):
            mps = mpsum_pool.tile([128, 512], F32, tag="mps", name=f"mps{b}")
            for cc in range(CCH):
                nc.tensor.matmul(mps[0:HID, 0:1], rw1t_t[cc][:],
                                 pooled_t[(b, cc)][:],
                                 start=(cc == 0), stop=(cc == CCH - 1))
            h_sb = small_pool.tile([HID, 1], F32, tag="h", name=f"h{b}")
            nc.scalar.activation(h_sb[:], mps[0:HID, 0:1], AF.Relu, bias=rb1_t[:])
            nc.tensor.matmul(mps[0:1, 4:4 + E], h_sb[:], rw2t_t[:],
                             start=True, stop=True)
            ze = small_pool.tile([1, E], F32, tag="ze", name=f"ze{b}")
            nc.vector.tensor_add(ze[:], mps[0:1, 4:4 + E], rb2_t[:])
            es = small_pool.tile([1, E], F32, tag="es", name=f"es{b}")
            nc.scalar.activation(es[:], ze[:], AF.Exp)
            ssum = small_pool.tile([1, 1], F32, tag="ssum", name=f"ss{b}")
            nc.vector.reduce_sum(out=ssum[:], in_=es[:], axis=AX.X)
            rec = small_pool.tile([1, 1], F32, tag="rec", name=f"rec{b}")
            nc.vector.reciprocal(rec[:], ssum[:])
            r_t = small_pool.tile([1, E], F32, tag="r", name=f"r{b}")
            nc.vector.tensor_scalar_mul(r_t[:], es[:], rec[:])
            # broadcast r across partitions: ones[1,128].T @ r[1,E] on PE
            nc.tensor.matmul(mps[0:128, 8:8 + E], ones_t[:], r_t[:],
                             start=True, stop=True)
            rbc = small_pool.tile([128, E], F32, tag="rbc", name=f"rbc{b}")
            nc.scalar.copy(rbc[:], mps[0:128, 8:8 + E])
            return rbc

        def emit_mixing(b, rbc):
            seg = KK * O          # per-expert block within a slab
            NG = 3                # ij-groups; PE can start after group 0
            gsz = seg // NG
            for cc in range(CCH):
                slab = slabs[cc]
                cmb = comb_pool.tile([128, seg], mmdt, tag="comb",
                                     name=f"cmb{b}_{cc}")
                for g in range(NG):
                    lo = g * gsz
                    a = scr_pool.tile([128, gsz], F32, tag="scr",
                                      name=f"scr{b}_{cc}_{g}")
                    nc.vector.tensor_scalar_mul(
                        a[:], slab[0][:, lo:lo + gsz], rbc[:, 0:1])
                    for e in range(1, E - 1):
                        nc.vector.scalar_tensor_tensor(
                            a[:], slab[e][:, lo:lo + gsz],
                            rbc[:, e:e + 1], a[:], op0=ALU.mult, op1=ALU.add)
                    nc.vector.scalar_tensor_tensor(
                        cmb[:, lo:lo + gsz],
                        slab[E - 1][:, lo:lo + gsz],
                        rbc[:, E - 1:E], a[:], op0=ALU.mult, op1=ALU.add)
                comb[(b, cc)] = cmb

        def emit_conv_ochunk(b, oc):
            ptiles = [cpsum_pool.tile([128, NBLK], F32, tag="cps",
                                      name=f"cp{b}_{oc}_{rb}")
                      for rb in range(NRB)]
            for cc in range(CCH):
                cmb = comb[(b, cc)]
                xvc = xv[(b, cc)]
                for ij in range(KK):
                    di, dj = ij // K, ij % K
                    w_ap = cmb[:, ij * O + oc * 128: ij * O + oc * 128 + 128]
                    first = (cc == 0 and ij == 0)
                    last = (cc == CCH - 1 and ij == KK - 1)
                    for rb in range(NRB):
                        rhs = xvc[:, rb * RB_ROWS + di: rb * RB_ROWS + di + RB_ROWS,
                                  dj: dj + W]
                        nc.tensor.matmul(ptiles[rb][:], w_ap, rhs,
                                         start=first, stop=last)
            for rb in range(NRB):
                st = ostg_pool.tile([128, NBLK], F32, tag="ostg",
                                    name=f"st{b}_{oc}_{rb}")
                nc.scalar.copy(st[:], ptiles[rb][:])
                nc.sync.dma_start(
                    out_d[b, oc * 128:(oc + 1) * 128,
                          rb * RB_ROWS:(rb + 1) * RB_ROWS, :],
                    st[:])

        # ---- emission: software-pipelined across samples ----
        def emit_pipeline():
            emit_loads(0)
            emit_slab_loads()
            rbc0 = emit_routing(0)
            emit_mixing(0, rbc0)
            pend_rbc = {}
            for b in range(BL):
                if b + 1 < BL:
                    emit_loads(b + 1)
                emit_conv_ochunk(b, 0)
                if b + 1 < BL:
                    pend_rbc[b + 1] = emit_routing(b + 1)
                emit_conv_ochunk(b, 1)
                if b + 1 < BL:
                    emit_mixing(b + 1, pend_rbc[b + 1])

        if loop_n is not None:
            # on-device HW loop around the whole pipeline (for timing)
            with tc.For_i(0, loop_n, 1):
                emit_pipeline()
        else:
            for _rep in range(reps):
                emit_pipeline()

    nc.compile()
    return nc


def _prep_inputs(x, experts, rw1, rb1, rw2, rb2):
    x = np.asarray(x, dtype=np.float32)
    x = np.ascontiguousarray(
        np.pad(x, ((0, 0), (0, 0), (1, 1), (1, 1))))
    experts = np.asarray(experts, dtype=np.float32)
    # [E,O,C,K,K] -> [C,E,K*K,O]
    ex_t = np.ascontiguousarray(
        np.transpose(experts, (2, 0, 3, 4, 1)).reshape(C, E, KK, O))
    rw1t = (np.asarray(rw1, dtype=np.float32) / float(H * W)).T  # [C, HID]
    rb1v = np.asarray(rb1, dtype=np.float32)
    rw2t = np.asarray(rw2, dtype=np.float32).T                   # [HID, E]
    rb2v = np.asarray(rb2, dtype=np.float32)
    RP = 2 * HID + 1 + 2 * E
    rp = np.zeros((128, RP), np.float32)
    rp[:, 0:HID] = rw1t[0:128]
    rp[:, HID:2 * HID] = rw1t[128:256]
    rp[0:HID, 2 * HID] = rb1v
    rp[0:HID, 2 * HID + 1:2 * HID + 1 + E] = rw2t
    rp[0, 2 * HID + 1 + E:2 * HID + 1 + 2 * E] = rb2v
    in_maps = []
    for i in range(NCORES):
        in_maps.append({
            "x": np.ascontiguousarray(x[i * BL:(i + 1) * BL]),
            "experts_t": ex_t,
            "rparams": rp,
        })
    return in_maps


def run(inputs, trace=False, use_f32r=True, **trace_kwargs):
    """Build (cached), run on 8 cores, return (full_out, BassKernelResults)."""
    key = ("prog", use_f32r)
    if key not in _CACHE:
        _CACHE[key] = _build_program(use_f32r=use_f32r)
    nc = _CACHE[key]
    in_maps = _prep_inputs(**inputs)
    res = run_bass_kernel_spmd(nc, in_maps, list(range(NCORES)),
                               trace=trace, **trace_kwargs)
    out = np.concatenate([res.results[i]["out"] for i in range(NCORES)], axis=0)
    return out, res


def kernel(x, experts, rw1, rb1, rw2, rb2):
    out, _ = run(dict(x=x, experts=experts, rw1=rw1, rb1=rb1, rw2=rw2, rb2=rb2))
    return out
